# revision 28
# baseline (speedup 1.0000x reference)
"""Trainium2 Bass kernel for nn_Decoder (gnn_message_passing).

Mathematical simplification of the reference (verified exact vs the jax oracle):
the reference's inner scan collapses — only the immediate predecessor (idx-1)
contributes to message aggregation, hv_new is invariant across inner steps, and
edge decisions reduce to per-node dot products d1[j]=hv_j.w1, d2[j]=hv_j.w2
thresholded at sigmoid>=0.5.

Per outer step idx (batch-on-partitions layout, 64 batch rows/core):
  enc[idx] = softmax(gs @ Wvert.T + bvert)          (gs = hv_{idx-1}, gs0 = z@Wlin1.T+blin1)
  a        = dep[:,idx,idx-1] * hv_{idx-1}           (idx>=1)
  h_in     = 15*sigmoid(b_gate)*b_map + sigmoid(a@Wg.T+bg) * (a@Wm.T+bm)   (h_in=gs0 at idx=0)
  gru gates from h_in and x_idx -> hv_idx
  d1[idx] = hv.w1, d2[idx] = hv.w2
edges[i,j] = step(d1[i]+d2[j]+be) for j<=i-2; edges[i,i-1] = step(d1[i-1]+d2[i-1]+be).

All matmuls run in fp32 (fp32r is an 11-bit-mantissa format — too coarse for the
hard edge thresholds). Biases enter via a constant ones-row appended to the
stationary operand (row 117 of the K-tile-3 slice) and bias rows baked into the
weight layouts, so no separate bias matmuls are needed.

Sharding: pure data parallel, batch 512 -> 64 per core across 8 cores.
"""

import numpy as np

B, S, H, C = 512, 16, 501, 7
NCORES = 8
BC = B // NCORES  # 64 batch rows per core
KT = [128, 128, 128, 117]    # K tiles over H=501 (data rows)
KTL = [128, 128, 128, 125]   # lhsT/rhs rows (tile 3: 117 data + ones row + 7 x rows)
KOFF = [0, 128, 256, 384]
H3 = 3 * H  # 1503

# packed bias/constant row: b_gate | b_map | b_edge | b_ih[0:2H] | b_hh[0:2H] | b_ih_n
BO_GATE, BO_MAP, BO_BE = 0, H, 2 * H
BO_IHRU = 2 * H + 1
BO_HHRU = BO_IHRU + 2 * H
BO_IHN = BO_HHRU + 2 * H
BIAS_LEN = BO_IHN + H

_CACHE = {}


def _build_module():
    import concourse.bass as bass
    import concourse.bacc as bacc
    import concourse.mybir as mybir
    from concourse.tile import TileContext

    f32 = mybir.dt.float32
    Alu = mybir.AluOpType
    Act = mybir.ActivationFunctionType
    Axis = mybir.AxisListType

    nc = bacc.Bacc("TRN2", target_bir_lowering=False, debug=False,
                   enable_asserts=False, num_devices=NCORES)

    # ---- DRAM I/O ----
    d_zT4 = nc.dram_tensor("zT4", [128, 256], f32, kind="ExternalInput")
    d_wlin = nc.dram_tensor("WlinT", [128, 4 * H], f32, kind="ExternalInput")
    d_wg = nc.dram_tensor("WgT", [128, 4 * H], f32, kind="ExternalInput")
    d_wm = nc.dram_tensor("WmT", [128, 4 * H], f32, kind="ExternalInput")
    d_whh = nc.dram_tensor("WhhT", [128, 4 * H3], f32, kind="ExternalInput")
    d_wih = nc.dram_tensor("WihT", [8, H3], f32, kind="ExternalInput")
    d_wv = nc.dram_tensor("WvT", [128, 4 * C], f32, kind="ExternalInput")
    d_w12 = nc.dram_tensor("w12", [128, 8], f32, kind="ExternalInput")
    d_xT = nc.dram_tensor("xT", [8, S * BC], f32, kind="ExternalInput")
    d_s4 = nc.dram_tensor("S4r", [128, 15 * 256], f32, kind="ExternalInput")
    d_eye = nc.dram_tensor("eye64", [64, 64], f32, kind="ExternalInput")
    d_mask = nc.dram_tensor("maskOD", [64, 256], f32, kind="ExternalInput")
    d_bias = nc.dram_tensor("BIASROW", [1, BIAS_LEN], f32, kind="ExternalInput")
    d_ones = nc.dram_tensor("onesrow", [1, 64], f32, kind="ExternalInput")
    d_zero8 = nc.dram_tensor("zero8", [8, 64], f32, kind="ExternalInput")
    d_odep = nc.dram_tensor("out_dep", [BC, 256], f32, kind="ExternalOutput")
    d_oenc = nc.dram_tensor("out_enc", [BC, S * C], f32, kind="ExternalOutput")

    def bcast(dram_handle, col0, ncols, nparts):
        ap = dram_handle.ap()
        return bass.AP(tensor=ap.tensor, offset=ap.offset + col0,
                       ap=[[0, nparts], [1, ncols]])

    with TileContext(nc) as tc:
        with (
            tc.tile_pool(name="const", bufs=1) as cp,
            tc.tile_pool(name="work", bufs=2) as wp,
            tc.tile_pool(name="psum", bufs=1, space="PSUM") as pp,
        ):
            # ---- constants into SBUF ----
            t_wlin = cp.tile([128, 4 * H], f32, name="t_wlin")
            t_wg = cp.tile([128, 4 * H], f32, name="t_wg")
            t_wm = cp.tile([128, 4 * H], f32, name="t_wm")
            t_whh = cp.tile([128, 4 * H3], f32, name="t_whh")
            t_wih = cp.tile([8, H3], f32, name="t_wih")
            t_wv = cp.tile([128, 4 * C], f32, name="t_wv")
            t_w12 = cp.tile([128, 8], f32, name="t_w12")
            t_xT = cp.tile([8, S * BC], f32, name="t_xT")
            t_s4 = cp.tile([128, 15 * 256], f32, name="t_s4")
            t_zT4 = cp.tile([128, 256], f32, name="t_zT4")
            t_eye = cp.tile([64, 64], f32, name="t_eye")
            t_mask = cp.tile([64, 256], f32, name="t_mask")
            t_bias = cp.tile([1, BIAS_LEN], f32, name="t_bias")
            t_c15 = cp.tile([64, H], f32, name="t_c15")
            t_nbe = cp.tile([64, 1], f32, name="t_nbe")
            t_enc = cp.tile([BC, S * C], f32, name="t_enc")
            t_d1 = cp.tile([64, 16], f32, name="t_d1")
            t_d2 = cp.tile([64, 16], f32, name="t_d2")
            t_ed = cp.tile([64, 256], f32, name="t_ed")
            t_th = cp.tile([64, 256], f32, name="t_th")
            t_sd = cp.tile([64, 16], f32, name="t_sd")
            t_bg = cp.tile([64, H], f32, name="t_bg")
            t_bm = cp.tile([64, H], f32, name="t_bm")
            t_bet = cp.tile([64, 1], f32, name="t_bet")
            t_bsc = cp.tile([1, 2 * H], f32, name="t_bsc")
            # persistent transposed-activation buffers (row 117 of slice 3 = ones)
            t_gsA = cp.tile([128, 256], f32, name="t_gsA")
            t_gsB = cp.tile([128, 256], f32, name="t_gsB")
            t_gsC = cp.tile([128, 256], f32, name="t_gsC")
            t_hT = cp.tile([128, 256], f32, name="t_hT")
            t_aT = cp.tile([128, 256], f32, name="t_aT")

            # Big weight streams on the sync (HWDGE) queue, in first-use order.
            nc.sync.dma_start(out=t_zT4[:, :], in_=d_zT4.ap())
            for k in range(4):
                nc.sync.dma_start(out=t_wlin[:, H * k:H * (k + 1)],
                                  in_=d_wlin.ap()[:, H * k:H * (k + 1)])
            for k in range(4):
                nc.sync.dma_start(out=t_whh[:, H3 * k:H3 * (k + 1)],
                                  in_=d_whh.ap()[:, H3 * k:H3 * (k + 1)])
            # step-0's aT only needs the first 256-col slice of S4r
            nc.sync.dma_start(out=t_s4[:, 0:256], in_=d_s4.ap()[:, 0:256])
            nc.sync.dma_start(out=t_wg[:, :], in_=d_wg.ap())
            nc.sync.dma_start(out=t_wm[:, :], in_=d_wm.ap())
            nc.sync.dma_start(out=t_s4[:, 256:], in_=d_s4.ap()[:, 256:])
            # Small setup transfers go on the gpsimd (SWDGE) queue so their
            # per-issue overhead doesn't serialize behind 8MB of weights.
            nc.gpsimd.dma_start(out=t_eye[:, :], in_=d_eye.ap())
            # bias + broadcasts first: the C15 sigmoid below absorbs the one-time
            # ACT table load (~2.7us) and should fire as early as possible
            nc.gpsimd.dma_start(out=t_bias[:, :], in_=d_bias.ap())
            nc.gpsimd.dma_start(out=t_bg[:, :], in_=bcast(d_bias, BO_GATE, H, 64))
            nc.gpsimd.dma_start(out=t_bm[:, :], in_=bcast(d_bias, BO_MAP, H, 64))
            # gsA doubles as hT at idx 0: its ones/x_0 rows gate step-0's
            # K-tile-3 matmuls, so patch them first
            nc.gpsimd.dma_start(out=t_gsA[117:118, 192:256], in_=d_ones.ap())
            nc.gpsimd.dma_start(out=t_gsA[118:125, 192:256], in_=d_xT.ap()[0:7, 0:64])
            nc.gpsimd.dma_start(out=t_gsA[125:126, 192:256], in_=d_zero8.ap()[0:1, :])
            nc.gpsimd.dma_start(out=t_xT[:, :], in_=d_xT.ap())
            nc.gpsimd.dma_start(out=t_wih[:, :], in_=d_wih.ap())
            for t in (t_gsB, t_gsC, t_hT, t_aT):
                nc.gpsimd.dma_start(out=t[117:118, 192:256], in_=d_ones.ap())
            for t in (t_gsB, t_gsC, t_aT):
                nc.gpsimd.dma_start(out=t[118:126, 192:256], in_=d_zero8.ap())
            nc.gpsimd.dma_start(out=t_bet[:, :], in_=bcast(d_bias, BO_BE, 1, 64))
            nc.gpsimd.dma_start(out=t_wv[:, :], in_=d_wv.ap())
            nc.gpsimd.dma_start(out=t_w12[:, :], in_=d_w12.ap())
            nc.gpsimd.dma_start(out=t_mask[:, :], in_=d_mask.ap())

            nc.vector.memset(t_d1[:, :], 0.0)
            nc.vector.memset(t_d2[:, :], 0.0)

            # fold b_ih(r,u)+b_hh(r,u) into the hh-weight bias row (row 117 of K-tile 3)
            nc.vector.tensor_tensor(out=t_bsc[0:1, :], in0=t_bias[0:1, BO_IHRU:BO_IHRU + 2 * H],
                                    in1=t_bias[0:1, BO_HHRU:BO_HHRU + 2 * H], op=Alu.add)
            nc.scalar.dma_start(out=t_whh[117:118, 3 * H3:3 * H3 + 2 * H], in_=t_bsc[0:1, :])


            # C15 = 15*sigmoid(b_gate)*b_map  (broadcast over 64 partitions)
            sg0 = wp.tile([64, H], f32, tag="sg")
            nc.scalar.activation(out=sg0[:, :], in_=t_bg[:, :], func=Act.Sigmoid)
            nc.vector.tensor_tensor(out=t_c15[:, :], in0=sg0[:, :], in1=t_bm[:, :], op=Alu.mult)
            nc.vector.tensor_scalar(out=t_c15[:, :], in0=t_c15[:, :],
                                    scalar1=float(S - 1), scalar2=None, op0=Alu.mult)
            # nbe = -b_edge - 1e-7 (threshold incl. the f32 sigmoid-rounding window)
            nc.vector.tensor_scalar(out=t_nbe[:, :], in0=t_bet[:, :],
                                    scalar1=-1.0, scalar2=-1e-7, op0=Alu.mult, op1=Alu.add)

            def mm_group(psum_ap, pairs):
                for i, (l, r) in enumerate(pairs):
                    nc.tensor.matmul(psum_ap, l, r,
                                     start=(i == 0), stop=(i == len(pairs) - 1))

            def transpose_into(psum_t, src, dst):
                # dst: (128,256) persistent sbuf; writes rows 0:117 of slice3 only
                for k in range(4):
                    nc.tensor.transpose(psum_t[0:KT[k], 64 * k:64 * k + 64],
                                        src[:, KOFF[k]:KOFF[k] + KT[k]], t_eye[:, :])
                nc.vector.tensor_copy(out=dst[0:128, 0:192], in_=psum_t[0:128, 0:192])
                nc.vector.tensor_copy(out=dst[0:117, 192:256], in_=psum_t[0:117, 192:256])

            def lhs_sl(buf, k):
                return buf[0:KTL[k], 64 * k:64 * k + 64]

            def hh_pairs(hT, g):
                return [(lhs_sl(hT, k),
                         t_whh[0:KTL[k], H3 * k + H * g:H3 * k + H * g + H]) for k in range(4)]

            def w_pairs(buf, t_w, n):
                return [(lhs_sl(buf, k), t_w[0:KTL[k], n * k:n * k + n]) for k in range(4)]

            gs_bufs = [t_gsA, t_gsB, t_gsC]

            def emit_enc(eidx, enc_src):
                plog = pp.tile([64, 16], f32, tag="psml")
                mm_group(plog[:, 0:C], w_pairs(enc_src, t_wv, C))
                mx = wp.tile([64, 1], f32, tag="mx")
                nc.vector.tensor_reduce(out=mx[:, :], in_=plog[:, 0:C], axis=Axis.X, op=Alu.max)
                nc.vector.tensor_scalar(out=mx[:, :], in0=mx[:, :],
                                        scalar1=-0.5, scalar2=None, op0=Alu.mult)
                th = wp.tile([64, C], f32, tag="th")
                nc.scalar.activation(out=th[:, :], in_=plog[:, 0:C], func=Act.Tanh,
                                     bias=mx[:, :], scale=0.5)
                num = wp.tile([64, C], f32, tag="num")
                nc.vector.tensor_scalar(out=num[:, :], in0=th[:, :],
                                        scalar1=1.0, scalar2=None, op0=Alu.add)
                den = wp.tile([64, C], f32, tag="den")
                nc.vector.tensor_scalar(out=den[:, :], in0=th[:, :],
                                        scalar1=-1.0, scalar2=1.0, op0=Alu.mult, op1=Alu.add)
                rd = wp.tile([64, C], f32, tag="rd")
                nc.vector.reciprocal(out=rd[:, :], in_=den[:, :])
                ex = wp.tile([64, C], f32, tag="ex")
                nc.vector.tensor_tensor(out=ex[:, :], in0=num[:, :], in1=rd[:, :], op=Alu.mult)
                sm = wp.tile([64, 1], f32, tag="sm")
                nc.vector.tensor_reduce(out=sm[:, :], in_=ex[:, :], axis=Axis.X, op=Alu.add)
                rc = wp.tile([64, 1], f32, tag="rc")
                nc.vector.reciprocal(out=rc[:, :], in_=sm[:, :])
                nc.vector.tensor_scalar(out=t_enc[:, C * eidx:C * eidx + C], in0=ex[:, :],
                                        scalar1=rc[:, :], scalar2=None, op0=Alu.mult)

            pending_enc = None
            gsT_prev = None
            # K-chunk ranges of the (64,H) activations; halves: A=chunks 0,1  B=chunks 2,3
            CH = [(KOFF[k], KOFF[k] + KT[k]) for k in range(4)]
            HA, HB = 256, H - 256  # 256 + 245

            def half_of(k):  # (half_index, col offset within the half) for K-chunk k
                return (0, KOFF[k]) if k < 2 else (1, KOFF[k] - 256)

            def paired_half_mms(ptiles, lhs_pairs0, lhs_pairs1, wt0, wt1, n0=H, n1=H):
                # For each N-half h: interleave 4 k-mms of group0 (rows 0:64) with
                # group1 (rows 64:128) into ptiles[h]; weights sliced per half.
                for h, (c0, c1) in enumerate(((0, HA), (HA, H))):
                    w = c1 - c0
                    for k in range(4):
                        l0, r0 = lhs_pairs0[k]
                        nc.tensor.matmul(ptiles[h][0:64, 0:w], l0,
                                         wt0(k, c0, c1), start=(k == 0), stop=(k == 3),
                                         skip_group_check=True)
                        if lhs_pairs1 is not None:
                            l1, r1 = lhs_pairs1[k]
                            nc.tensor.matmul(ptiles[h][64:128, 0:w], l1,
                                             wt1(k, c0, c1), start=(k == 0), stop=(k == 3),
                                             skip_group_check=True)

            for idx in range(S):
                if idx == 0:
                    pgm = [pp.tile([128, 512], f32, tag="pgmA", name="pgmA"),
                           pp.tile([128, 512], f32, tag="pgmB", name="pgmB")]
                    for h, (c0, c1) in enumerate(((0, HA), (HA, H))):
                        w = c1 - c0
                        for k in range(4):
                            nc.tensor.matmul(pgm[h][0:64, 0:w],
                                             t_zT4[0:KTL[k], 64 * k:64 * k + 64],
                                             t_wlin[0:KTL[k], H * k + c0:H * k + c1],
                                             start=(k == 0), stop=(k == 3),
                                             skip_group_check=True)
                    h_in = wp.tile([64, H], f32, tag="h_in")
                    pt = pp.tile([128, 256], f32, tag="pT")
                    gsT = t_gsA
                    for k in range(4):
                        a, b = CH[k]
                        h, co = half_of(k)
                        nc.vector.tensor_copy(out=h_in[:, a:b],
                                              in_=pgm[h][0:64, co:co + KT[k]])
                        nc.tensor.transpose(pt[0:KT[k], 64 * k:64 * k + 64],
                                            h_in[:, a:b], t_eye[:, :])
                        nc.vector.tensor_copy(out=gsT[0:KT[k], 64 * k:64 * k + 64],
                                              in_=pt[0:KT[k], 64 * k:64 * k + 64])
                    hT = gsT  # h_in == gs at idx 0
                else:
                    gsT = None
                    pgm = [pp.tile([128, 512], f32, tag="pgmA", name="pgmA"),
                           pp.tile([128, 512], f32, tag="pgmB", name="pgmB")]
                    gp = w_pairs(t_aT, t_wg, H)
                    paired_half_mms(pgm, gp, gp,
                                    lambda k, c0, c1: t_wg[0:KTL[k], H * k + c0:H * k + c1],
                                    lambda k, c0, c1: t_wm[0:KTL[k], H * k + c0:H * k + c1])
                    # chunk-wavefront: each K-chunk of h_in flows to its hT slice as
                    # soon as its N-half's matmul group completes
                    sg = wp.tile([64, H], f32, tag="sg")
                    h_in = wp.tile([64, H], f32, tag="h_in")
                    pt = pp.tile([128, 256], f32, tag="pT")
                    hT = t_hT
                    for k in range(4):
                        a, b = CH[k]
                        h, co = half_of(k)
                        nc.scalar.activation(out=sg[:, a:b], in_=pgm[h][0:64, co:co + KT[k]],
                                             func=Act.Sigmoid)
                        nc.vector.tensor_tensor(out=h_in[:, a:b], in0=sg[:, a:b],
                                                in1=pgm[h][64:128, co:co + KT[k]], op=Alu.mult)
                        nc.vector.tensor_tensor(out=h_in[:, a:b], in0=h_in[:, a:b],
                                                in1=t_c15[:, a:b], op=Alu.add)
                        nc.tensor.transpose(pt[0:KT[k], 64 * k:64 * k + 64],
                                            h_in[:, a:b], t_eye[:, :])
                        nc.vector.tensor_copy(out=hT[0:KT[k], 64 * k:64 * k + 64],
                                              in_=pt[0:KT[k], 64 * k:64 * k + 64])

                # ---- GRU ----
                if idx > 0:
                    nc.scalar.dma_start(out=t_hT[118:125, 192:256],
                                        in_=t_xT[0:7, BC * idx:BC * idx + 64])
                prh = [pp.tile([128, 512], f32, tag="prhA", name="prhA"),
                       pp.tile([128, 512], f32, tag="prhB", name="prhB")]
                rp = hh_pairs(hT, 0)
                hp = hh_pairs(hT, 2)
                paired_half_mms(prh, rp, hp,
                                lambda k, c0, c1: t_whh[0:KTL[k], H3 * k + c0:H3 * k + c1],
                                lambda k, c0, c1: t_whh[0:KTL[k], H3 * k + 2 * H + c0:H3 * k + 2 * H + c1])
                pui = [pp.tile([128, 512], f32, tag="puiA", name="puiA"),
                       pp.tile([128, 512], f32, tag="puiB", name="puiB")]
                xsl = t_xT[0:8, BC * idx:BC * idx + 64]
                up = hh_pairs(hT, 1)
                for h, (c0, c1) in enumerate(((0, HA), (HA, H))):
                    w = c1 - c0
                    nc.tensor.matmul(pui[h][64:128, 0:w], xsl,
                                     t_wih[0:8, 2 * H + c0:2 * H + c1],
                                     start=True, stop=True, skip_group_check=True)
                    for k in range(4):
                        nc.tensor.matmul(pui[h][0:64, 0:w], up[k][0],
                                         t_whh[0:KTL[k], H3 * k + H + c0:H3 * k + H + c1],
                                         start=(k == 0), stop=(k == 3),
                                         skip_group_check=True)

                # merged GRU wave per K-chunk: r -> t1 -> t2 -> tanh -> d,
                # u -> hv = n + u*d, then transpose + aT-scale immediately.
                # Interleaving u with the tanh chain keeps sigma(u_k) from
                # queueing behind all four tanh ops on ACT.
                r = wp.tile([64, H], f32, tag="r")
                t1 = wp.tile([64, H], f32, tag="t1")
                t2 = wp.tile([64, H], f32, tag="t2")
                n = wp.tile([64, H], f32, tag="n")
                dd = wp.tile([64, H], f32, tag="dd")
                u = wp.tile([64, H], f32, tag="u")
                hv = wp.tile([64, H], f32, tag="hv")
                pt2 = pp.tile([128, 256], f32, tag="pT")
                gsT_new = gs_bufs[(idx + 1) % 3]
                for k in range(4):
                    a, b = CH[k]
                    h, co = half_of(k)
                    cs = slice(co, co + KT[k])
                    nc.scalar.activation(out=r[:, a:b], in_=prh[h][0:64, cs], func=Act.Sigmoid)
                    nc.vector.tensor_tensor(out=t1[:, a:b], in0=r[:, a:b],
                                            in1=prh[h][64:128, cs], op=Alu.mult)
                    nc.vector.tensor_tensor(out=t2[:, a:b], in0=t1[:, a:b],
                                            in1=pui[h][64:128, cs], op=Alu.add)
                    nc.scalar.activation(out=n[:, a:b], in_=t2[:, a:b], func=Act.Tanh)
                    nc.vector.tensor_tensor(out=dd[:, a:b], in0=h_in[:, a:b],
                                            in1=n[:, a:b], op=Alu.subtract)
                    nc.scalar.activation(out=u[:, a:b], in_=pui[h][0:64, cs], func=Act.Sigmoid)
                    nc.vector.tensor_tensor(out=hv[:, a:b], in0=u[:, a:b],
                                            in1=dd[:, a:b], op=Alu.mult)
                    nc.vector.tensor_tensor(out=hv[:, a:b], in0=hv[:, a:b],
                                            in1=n[:, a:b], op=Alu.add)
                    nc.tensor.transpose(pt2[0:KT[k], 64 * k:64 * k + 64],
                                        hv[:, a:b], t_eye[:, :])
                    nc.vector.tensor_copy(out=gsT_new[0:KT[k], 64 * k:64 * k + 64],
                                          in_=pt2[0:KT[k], 64 * k:64 * k + 64])
                    if idx + 1 < S:
                        s0 = 256 * idx
                        nc.vector.tensor_tensor(
                            out=t_aT[0:KT[k], 64 * k:64 * k + 64],
                            in0=pt2[0:KT[k], 64 * k:64 * k + 64],
                            in1=t_s4[0:KT[k], s0 + 64 * k:s0 + 64 * k + 64], op=Alu.mult)

                # ---- dots d1,d2 ----
                psm = pp.tile([64, 16], f32, tag="psml")
                mm_group(psm[:, 8:10], w_pairs(gsT_new, t_w12, 2))
                nc.vector.tensor_copy(out=t_d1[:, idx:idx + 1], in_=psm[:, 8:9])
                nc.vector.tensor_copy(out=t_d2[:, idx:idx + 1], in_=psm[:, 9:10])

                # ---- enc softmax: previous step's, deferred for priority ----
                if pending_enc is not None:
                    emit_enc(*pending_enc)
                pending_enc = (idx, gsT if idx == 0 else gsT_prev)

                # ---- edge row idx: margins -> threshold -> mask (progressive) ----
                nc.vector.tensor_scalar(out=t_ed[:, 16 * idx:16 * idx + 16], in0=t_d2[:, :],
                                        scalar1=t_d1[:, idx:idx + 1], scalar2=None, op0=Alu.add)
                nc.vector.tensor_scalar(out=t_th[:, 16 * idx:16 * idx + 16],
                                        in0=t_ed[:, 16 * idx:16 * idx + 16],
                                        scalar1=t_nbe[:, :], scalar2=None, op0=Alu.is_ge)
                nc.vector.tensor_tensor(out=t_th[:, 16 * idx:16 * idx + 16],
                                        in0=t_th[:, 16 * idx:16 * idx + 16],
                                        in1=t_mask[:, 16 * idx:16 * idx + 16], op=Alu.mult)
                if idx >= 1:
                    # diagonal entry tth[:, 17*idx-1] = step(d1[idx-1]+d2[idx-1]+be)
                    nc.vector.tensor_tensor(out=t_sd[:, idx - 1:idx],
                                            in0=t_d1[:, idx - 1:idx],
                                            in1=t_d2[:, idx - 1:idx], op=Alu.add)
                    nc.vector.tensor_scalar(out=t_th[:, 17 * idx - 1:17 * idx],
                                            in0=t_sd[:, idx - 1:idx],
                                            scalar1=t_nbe[:, :], scalar2=None, op0=Alu.is_ge)

                gsT_prev = gsT_new

            if pending_enc is not None:
                emit_enc(*pending_enc)

            # edges were finalized progressively inside the step loop
            nc.sync.dma_start(out=d_odep.ap(), in_=t_th[:, :])
            nc.sync.dma_start(out=d_oenc.ap(), in_=t_enc[:, :])

    nc.compile()
    return nc


def _host_prep(z, dep_graph, node_encoding, W_lin1, b_lin1, W_vert, b_vert,
               W_edge, b_edge, W_gate, b_gate, W_map, b_map, W_ih, b_ih, W_hh, b_hh):
    f = np.float32

    def ktiled(WT, bias=None):  # WT: (H, N) -> (128, 4N); bias lands at row 117 of tile 3
        N = WT.shape[1]
        out = np.zeros((128, 4 * N), f)
        for k in range(4):
            out[0:KT[k], N * k:N * (k + 1)] = WT[KOFF[k]:KOFF[k] + KT[k], :]
        if bias is not None:
            out[117, 3 * N:3 * N + N] = bias
        return out

    whhT = np.zeros((128, 4 * H3), f)
    whhsrc = W_hh.T.astype(f)  # (H, 3H)
    for k in range(4):
        whhT[0:KT[k], H3 * k:H3 * (k + 1)] = whhsrc[KOFF[k]:KOFF[k] + KT[k], :]
    whhT[117, 3 * H3:4 * H3] = b_hh  # r/u thirds get b_ih added on-device
    # spare rows 118:125 of K-tile 3 carry W_ih.T for the r/u thirds (x rows of lhsT)
    whhT[118:125, 3 * H3:3 * H3 + 2 * H] = W_ih.T[:, 0:2 * H]

    shared = {
        "WlinT": ktiled(W_lin1.T.astype(f), b_lin1),
        "WgT": ktiled(W_gate.T.astype(f), b_gate),
        "WmT": ktiled(W_map.T.astype(f), b_map),
        "WhhT": whhT,
        "WvT": ktiled(W_vert.T.astype(f), b_vert),
        "w12": ktiled(W_edge.reshape(2, H).T.astype(f)),
        "eye64": np.eye(64, dtype=f),
        "onesrow": np.ones((1, 64), f),
    }
    wih = np.zeros((8, H3), f)
    wih[0:7] = W_ih.T
    wih[7] = b_ih
    shared["WihT"] = wih
    mask = np.zeros((64, 256), f)
    for i in range(16):
        for j in range(16):
            if j <= i - 2:
                mask[:, 16 * i + j] = 1.0
    shared["maskOD"] = mask
    bias = np.zeros((1, BIAS_LEN), f)
    bias[0, BO_GATE:BO_GATE + H] = b_gate
    bias[0, BO_MAP:BO_MAP + H] = b_map
    bias[0, BO_BE] = np.asarray(b_edge).ravel()[0]
    bias[0, BO_IHRU:BO_IHRU + 2 * H] = b_ih[0:2 * H]
    bias[0, BO_HHRU:BO_HHRU + 2 * H] = b_hh[0:2 * H]
    bias[0, BO_IHN:BO_IHN + H] = b_ih[2 * H:3 * H]
    shared["BIASROW"] = bias
    shared["zero8"] = np.zeros((8, 64), f)

    in_maps = []
    for c in range(NCORES):
        sl = slice(c * BC, (c + 1) * BC)
        zc = np.asarray(z[sl], f)
        zt4 = np.zeros((128, 256), f)
        for k in range(4):
            zt4[0:KT[k], 64 * k:64 * k + 64] = zc[:, KOFF[k]:KOFF[k] + KT[k]].T
        zt4[117, 192:256] = 1.0  # ones row for b_lin1
        xt = np.zeros((8, S * BC), f)
        nec = np.asarray(node_encoding[sl], f)  # (64, S, C)
        for idx in range(S):
            xt[0:7, BC * idx:BC * (idx + 1)] = nec[:, idx, :].T
        xt[7] = 1.0
        depc = np.asarray(dep_graph[sl], f)
        s4 = np.zeros((128, 15 * 256), f)
        for i in range(1, S):
            sub = depc[:, i, i - 1]  # (64,)
            s4[:, 256 * (i - 1):256 * i] = np.tile(sub[None, :], (128, 4))
        m = dict(shared)
        m.update({"zT4": zt4, "xT": xt, "S4r": s4})
        in_maps.append(m)
    return in_maps


def kernel(**inputs):
    from concourse.bass_utils import run_bass_kernel_spmd

    if "nc" not in _CACHE:
        _CACHE["nc"] = _build_module()
    nc = _CACHE["nc"]
    in_maps = _host_prep(**inputs)
    res = run_bass_kernel_spmd(nc, in_maps, core_ids=list(range(NCORES)))
    dep_out = np.concatenate(
        [res.results[c]["out_dep"].reshape(BC, S, S) for c in range(NCORES)], axis=0)
    enc_out = np.concatenate(
        [res.results[c]["out_enc"].reshape(BC, S, C) for c in range(NCORES)], axis=0)
    return dep_out.astype(np.float32), enc_out.astype(np.float32)


# revision 29
# speedup vs baseline: 1.0689x; 1.0689x over previous
"""Trainium2 Bass kernel for nn_Decoder (gnn_message_passing).

Mathematical simplification of the reference (verified exact vs the jax oracle):
the reference's inner scan collapses — only the immediate predecessor (idx-1)
contributes to message aggregation, hv_new is invariant across inner steps, and
edge decisions reduce to per-node dot products d1[j]=hv_j.w1, d2[j]=hv_j.w2
thresholded at sigmoid>=0.5.

Per outer step idx (batch-on-partitions layout, 64 batch rows/core):
  enc[idx] = softmax(gs @ Wvert.T + bvert)          (gs = hv_{idx-1}, gs0 = z@Wlin1.T+blin1)
  a        = dep[:,idx,idx-1] * hv_{idx-1}           (idx>=1)
  h_in     = 15*sigmoid(b_gate)*b_map + sigmoid(a@Wg.T+bg) * (a@Wm.T+bm)   (h_in=gs0 at idx=0)
  gru gates from h_in and x_idx -> hv_idx
  d1[idx] = hv.w1, d2[idx] = hv.w2
edges[i,j] = step(d1[i]+d2[j]+be) for j<=i-2; edges[i,i-1] = step(d1[i-1]+d2[i-1]+be).

All matmuls run in fp32 (fp32r is an 11-bit-mantissa format — too coarse for the
hard edge thresholds). Biases enter via a constant ones-row appended to the
stationary operand (row 117 of the K-tile-3 slice) and bias rows baked into the
weight layouts, so no separate bias matmuls are needed.

Sharding: pure data parallel, batch 512 -> 64 per core across 8 cores.
"""

import numpy as np

B, S, H, C = 512, 16, 501, 7
NCORES = 8
BC = B // NCORES  # 64 batch rows per core
KT = [128, 128, 128, 117]    # K tiles over H=501 (data rows)
KTL = [128, 128, 128, 125]   # lhsT/rhs rows (tile 3: 117 data + ones row + 7 x rows)
KOFF = [0, 128, 256, 384]
H3 = 3 * H  # 1503

# packed bias/constant row: b_gate | b_map | b_edge | b_ih[0:2H] | b_hh[0:2H] | b_ih_n
BO_GATE, BO_MAP, BO_BE = 0, H, 2 * H
BO_IHRU = 2 * H + 1
BO_HHRU = BO_IHRU + 2 * H
BO_IHN = BO_HHRU + 2 * H
BIAS_LEN = BO_IHN + H

_CACHE = {}


def _build_module():
    import concourse.bass as bass
    import concourse.bacc as bacc
    import concourse.mybir as mybir
    from concourse.tile import TileContext

    f32 = mybir.dt.float32
    Alu = mybir.AluOpType
    Act = mybir.ActivationFunctionType
    Axis = mybir.AxisListType

    nc = bacc.Bacc("TRN2", target_bir_lowering=False, debug=False,
                   enable_asserts=False, num_devices=NCORES)

    # ---- DRAM I/O ----
    d_zT4 = nc.dram_tensor("zT4", [128, 256], f32, kind="ExternalInput")
    d_wlin = nc.dram_tensor("WlinT", [128, 4 * H], f32, kind="ExternalInput")
    d_wg = nc.dram_tensor("WgT", [128, 4 * H], f32, kind="ExternalInput")
    d_wm = nc.dram_tensor("WmT", [128, 4 * H], f32, kind="ExternalInput")
    d_whh = nc.dram_tensor("WhhT", [128, 4 * H3], f32, kind="ExternalInput")
    d_wih = nc.dram_tensor("WihT", [8, H3], f32, kind="ExternalInput")
    d_wv = nc.dram_tensor("WvT", [128, 4 * 9], f32, kind="ExternalInput")
    d_xT = nc.dram_tensor("xT", [8, S * BC], f32, kind="ExternalInput")
    d_s4 = nc.dram_tensor("S4r", [128, 15 * 256], f32, kind="ExternalInput")
    d_eye = nc.dram_tensor("eye64", [64, 64], f32, kind="ExternalInput")
    d_mask = nc.dram_tensor("maskOD", [64, 256], f32, kind="ExternalInput")
    d_bias = nc.dram_tensor("BIASROW", [1, BIAS_LEN], f32, kind="ExternalInput")
    d_ones = nc.dram_tensor("onesrow", [1, 64], f32, kind="ExternalInput")
    d_zero8 = nc.dram_tensor("zero8", [8, 64], f32, kind="ExternalInput")
    d_odep = nc.dram_tensor("out_dep", [BC, 256], f32, kind="ExternalOutput")
    d_oenc = nc.dram_tensor("out_enc", [BC, S * C], f32, kind="ExternalOutput")

    def bcast(dram_handle, col0, ncols, nparts):
        ap = dram_handle.ap()
        return bass.AP(tensor=ap.tensor, offset=ap.offset + col0,
                       ap=[[0, nparts], [1, ncols]])

    with TileContext(nc) as tc:
        with (
            tc.tile_pool(name="const", bufs=1) as cp,
            tc.tile_pool(name="work", bufs=2) as wp,
            tc.tile_pool(name="psum", bufs=1, space="PSUM") as pp,
        ):
            # ---- constants into SBUF ----
            t_wlin = cp.tile([128, 4 * H], f32, name="t_wlin")
            t_wg = cp.tile([128, 4 * H], f32, name="t_wg")
            t_wm = cp.tile([128, 4 * H], f32, name="t_wm")
            t_whh = cp.tile([128, 4 * H3], f32, name="t_whh")
            t_wih = cp.tile([8, H3], f32, name="t_wih")
            t_wv = cp.tile([128, 4 * 9], f32, name="t_wv")
            t_xT = cp.tile([8, S * BC], f32, name="t_xT")
            t_s4 = cp.tile([128, 15 * 256], f32, name="t_s4")
            t_zT4 = cp.tile([128, 256], f32, name="t_zT4")
            t_eye = cp.tile([64, 64], f32, name="t_eye")
            t_mask = cp.tile([64, 256], f32, name="t_mask")
            t_bias = cp.tile([1, BIAS_LEN], f32, name="t_bias")
            t_c15 = cp.tile([64, H], f32, name="t_c15")
            t_nbe = cp.tile([64, 1], f32, name="t_nbe")
            t_enc = cp.tile([BC, S * C], f32, name="t_enc")
            t_d1 = cp.tile([64, 16], f32, name="t_d1")
            t_d2 = cp.tile([64, 16], f32, name="t_d2")
            t_ed = cp.tile([64, 256], f32, name="t_ed")
            t_th = cp.tile([64, 256], f32, name="t_th")
            t_sd = cp.tile([64, 16], f32, name="t_sd")
            t_bg = cp.tile([64, H], f32, name="t_bg")
            t_bm = cp.tile([64, H], f32, name="t_bm")
            t_bet = cp.tile([64, 1], f32, name="t_bet")
            t_bsc = cp.tile([1, 2 * H], f32, name="t_bsc")
            # persistent transposed-activation buffers (row 117 of slice 3 = ones)
            t_gsA = cp.tile([128, 256], f32, name="t_gsA")
            t_gsB = cp.tile([128, 256], f32, name="t_gsB")
            t_gsC = cp.tile([128, 256], f32, name="t_gsC")
            t_hT = cp.tile([128, 256], f32, name="t_hT")
            t_aT = cp.tile([128, 256], f32, name="t_aT")

            # Big weight streams on the sync (HWDGE) queue, in first-use order.
            nc.sync.dma_start(out=t_zT4[:, :], in_=d_zT4.ap())
            for k in range(4):
                nc.sync.dma_start(out=t_wlin[:, H * k:H * (k + 1)],
                                  in_=d_wlin.ap()[:, H * k:H * (k + 1)])
            for k in range(4):
                nc.sync.dma_start(out=t_whh[:, H3 * k:H3 * (k + 1)],
                                  in_=d_whh.ap()[:, H3 * k:H3 * (k + 1)])
            # step-0's aT only needs the first 256-col slice of S4r
            nc.sync.dma_start(out=t_s4[:, 0:256], in_=d_s4.ap()[:, 0:256])
            nc.sync.dma_start(out=t_wg[:, :], in_=d_wg.ap())
            nc.sync.dma_start(out=t_wm[:, :], in_=d_wm.ap())
            nc.sync.dma_start(out=t_s4[:, 256:], in_=d_s4.ap()[:, 256:])
            # Small setup transfers go on the gpsimd (SWDGE) queue so their
            # per-issue overhead doesn't serialize behind 8MB of weights.
            nc.gpsimd.dma_start(out=t_eye[:, :], in_=d_eye.ap())
            # bias + broadcasts first: the C15 sigmoid below absorbs the one-time
            # ACT table load (~2.7us) and should fire as early as possible
            nc.gpsimd.dma_start(out=t_bias[:, :], in_=d_bias.ap())
            nc.gpsimd.dma_start(out=t_bg[:, :], in_=bcast(d_bias, BO_GATE, H, 64))
            nc.gpsimd.dma_start(out=t_bm[:, :], in_=bcast(d_bias, BO_MAP, H, 64))
            # gsA doubles as hT at idx 0: its ones/x_0 rows gate step-0's
            # K-tile-3 matmuls, so patch them first
            nc.gpsimd.dma_start(out=t_gsA[117:118, 192:256], in_=d_ones.ap())
            nc.gpsimd.dma_start(out=t_gsA[118:125, 192:256], in_=d_xT.ap()[0:7, 0:64])
            nc.gpsimd.dma_start(out=t_gsA[125:126, 192:256], in_=d_zero8.ap()[0:1, :])
            nc.gpsimd.dma_start(out=t_xT[:, :], in_=d_xT.ap())
            nc.gpsimd.dma_start(out=t_wih[:, :], in_=d_wih.ap())
            for t in (t_gsB, t_gsC, t_hT, t_aT):
                nc.gpsimd.dma_start(out=t[117:118, 192:256], in_=d_ones.ap())
            for t in (t_gsB, t_gsC, t_aT):
                nc.gpsimd.dma_start(out=t[118:126, 192:256], in_=d_zero8.ap())
            nc.gpsimd.dma_start(out=t_bet[:, :], in_=bcast(d_bias, BO_BE, 1, 64))
            nc.gpsimd.dma_start(out=t_wv[:, :], in_=d_wv.ap())
            nc.gpsimd.dma_start(out=t_mask[:, :], in_=d_mask.ap())

            nc.vector.memset(t_d1[:, :], 0.0)
            nc.vector.memset(t_d2[:, :], 0.0)

            # fold b_ih(r,u)+b_hh(r,u) into the hh-weight bias row (row 117 of K-tile 3)
            nc.vector.tensor_tensor(out=t_bsc[0:1, :], in0=t_bias[0:1, BO_IHRU:BO_IHRU + 2 * H],
                                    in1=t_bias[0:1, BO_HHRU:BO_HHRU + 2 * H], op=Alu.add)
            nc.scalar.dma_start(out=t_whh[117:118, 3 * H3:3 * H3 + 2 * H], in_=t_bsc[0:1, :])


            # C15 = 15*sigmoid(b_gate)*b_map  (broadcast over 64 partitions)
            sg0 = wp.tile([64, H], f32, tag="sg")
            nc.scalar.activation(out=sg0[:, :], in_=t_bg[:, :], func=Act.Sigmoid)
            nc.vector.tensor_tensor(out=t_c15[:, :], in0=sg0[:, :], in1=t_bm[:, :], op=Alu.mult)
            nc.vector.tensor_scalar(out=t_c15[:, :], in0=t_c15[:, :],
                                    scalar1=float(S - 1), scalar2=None, op0=Alu.mult)
            # nbe = -b_edge - 1e-7 (threshold incl. the f32 sigmoid-rounding window)
            nc.vector.tensor_scalar(out=t_nbe[:, :], in0=t_bet[:, :],
                                    scalar1=-1.0, scalar2=-1e-7, op0=Alu.mult, op1=Alu.add)

            def mm_group(psum_ap, pairs):
                for i, (l, r) in enumerate(pairs):
                    nc.tensor.matmul(psum_ap, l, r,
                                     start=(i == 0), stop=(i == len(pairs) - 1))

            def transpose_into(psum_t, src, dst):
                # dst: (128,256) persistent sbuf; writes rows 0:117 of slice3 only
                for k in range(4):
                    nc.tensor.transpose(psum_t[0:KT[k], 64 * k:64 * k + 64],
                                        src[:, KOFF[k]:KOFF[k] + KT[k]], t_eye[:, :])
                nc.vector.tensor_copy(out=dst[0:128, 0:192], in_=psum_t[0:128, 0:192])
                nc.vector.tensor_copy(out=dst[0:117, 192:256], in_=psum_t[0:117, 192:256])

            def lhs_sl(buf, k):
                return buf[0:KTL[k], 64 * k:64 * k + 64]

            def hh_pairs(hT, g):
                return [(lhs_sl(hT, k),
                         t_whh[0:KTL[k], H3 * k + H * g:H3 * k + H * g + H]) for k in range(4)]

            def w_pairs(buf, t_w, n):
                return [(lhs_sl(buf, k), t_w[0:KTL[k], n * k:n * k + n]) for k in range(4)]

            gs_bufs = [t_gsA, t_gsB, t_gsC]

            def emit_enc(eidx, plog):
                mx = wp.tile([64, 1], f32, tag="mx")
                nc.vector.tensor_reduce(out=mx[:, :], in_=plog[:, 0:C], axis=Axis.X, op=Alu.max)
                nc.vector.tensor_scalar(out=mx[:, :], in0=mx[:, :],
                                        scalar1=-0.5, scalar2=None, op0=Alu.mult)
                th = wp.tile([64, C], f32, tag="th")
                nc.scalar.activation(out=th[:, :], in_=plog[:, 0:C], func=Act.Tanh,
                                     bias=mx[:, :], scale=0.5)
                num = wp.tile([64, C], f32, tag="num")
                nc.vector.tensor_scalar(out=num[:, :], in0=th[:, :],
                                        scalar1=1.0, scalar2=None, op0=Alu.add)
                den = wp.tile([64, C], f32, tag="den")
                nc.vector.tensor_scalar(out=den[:, :], in0=th[:, :],
                                        scalar1=-1.0, scalar2=1.0, op0=Alu.mult, op1=Alu.add)
                rd = wp.tile([64, C], f32, tag="rd")
                nc.vector.reciprocal(out=rd[:, :], in_=den[:, :])
                ex = wp.tile([64, C], f32, tag="ex")
                nc.vector.tensor_tensor(out=ex[:, :], in0=num[:, :], in1=rd[:, :], op=Alu.mult)
                sm = wp.tile([64, 1], f32, tag="sm")
                nc.vector.tensor_reduce(out=sm[:, :], in_=ex[:, :], axis=Axis.X, op=Alu.add)
                rc = wp.tile([64, 1], f32, tag="rc")
                nc.vector.reciprocal(out=rc[:, :], in_=sm[:, :])
                nc.vector.tensor_scalar(out=t_enc[:, C * eidx:C * eidx + C], in0=ex[:, :],
                                        scalar1=rc[:, :], scalar2=None, op0=Alu.mult)

            pending_enc = None
            gsT_prev = None
            # K-chunk ranges of the (64,H) activations; halves: A=chunks 0,1  B=chunks 2,3
            CH = [(KOFF[k], KOFF[k] + KT[k]) for k in range(4)]
            HA, HB = 256, H - 256  # 256 + 245

            def half_of(k):  # (half_index, col offset within the half) for K-chunk k
                return (0, KOFF[k]) if k < 2 else (1, KOFF[k] - 256)

            def paired_half_mms(ptiles, lhs_pairs0, lhs_pairs1, wt0, wt1, n0=H, n1=H):
                # For each N-half h: interleave 4 k-mms of group0 (rows 0:64) with
                # group1 (rows 64:128) into ptiles[h]; weights sliced per half.
                for h, (c0, c1) in enumerate(((0, HA), (HA, H))):
                    w = c1 - c0
                    for k in range(4):
                        l0, r0 = lhs_pairs0[k]
                        nc.tensor.matmul(ptiles[h][0:64, 0:w], l0,
                                         wt0(k, c0, c1), start=(k == 0), stop=(k == 3),
                                         skip_group_check=True)
                        if lhs_pairs1 is not None:
                            l1, r1 = lhs_pairs1[k]
                            nc.tensor.matmul(ptiles[h][64:128, 0:w], l1,
                                             wt1(k, c0, c1), start=(k == 0), stop=(k == 3),
                                             skip_group_check=True)

            for idx in range(S):
                if idx == 0:
                    pgm = [pp.tile([128, 512], f32, tag="pgmA", name="pgmA"),
                           pp.tile([128, 512], f32, tag="pgmB", name="pgmB")]
                    for h, (c0, c1) in enumerate(((0, HA), (HA, H))):
                        w = c1 - c0
                        for k in range(4):
                            nc.tensor.matmul(pgm[h][0:64, 0:w],
                                             t_zT4[0:KTL[k], 64 * k:64 * k + 64],
                                             t_wlin[0:KTL[k], H * k + c0:H * k + c1],
                                             start=(k == 0), stop=(k == 3),
                                             skip_group_check=True)
                    h_in = wp.tile([64, H], f32, tag="h_in")
                    pt = pp.tile([128, 256], f32, tag="pT")
                    gsT = t_gsA
                    for k in range(4):
                        a, b = CH[k]
                        h, co = half_of(k)
                        nc.vector.tensor_copy(out=h_in[:, a:b],
                                              in_=pgm[h][0:64, co:co + KT[k]])
                        nc.tensor.transpose(pt[0:KT[k], 64 * k:64 * k + 64],
                                            h_in[:, a:b], t_eye[:, :])
                        nc.vector.tensor_copy(out=gsT[0:KT[k], 64 * k:64 * k + 64],
                                              in_=pt[0:KT[k], 64 * k:64 * k + 64])
                    hT = gsT  # h_in == gs at idx 0
                    psm0 = pp.tile([64, 16], f32, tag="psml")
                    mm_group(psm0[:, 0:9], w_pairs(gsT, t_wv, 9))
                    pending_enc = (0, psm0)
                else:
                    gsT = None
                    pgm = [pp.tile([128, 512], f32, tag="pgmA", name="pgmA"),
                           pp.tile([128, 512], f32, tag="pgmB", name="pgmB")]
                    gp = w_pairs(t_aT, t_wg, H)
                    paired_half_mms(pgm, gp, gp,
                                    lambda k, c0, c1: t_wg[0:KTL[k], H * k + c0:H * k + c1],
                                    lambda k, c0, c1: t_wm[0:KTL[k], H * k + c0:H * k + c1])
                    # chunk-wavefront: each K-chunk of h_in flows to its hT slice as
                    # soon as its N-half's matmul group completes
                    sg = wp.tile([64, H], f32, tag="sg")
                    h_in = wp.tile([64, H], f32, tag="h_in")
                    pt = pp.tile([128, 256], f32, tag="pT")
                    hT = t_hT
                    for k in range(4):
                        a, b = CH[k]
                        h, co = half_of(k)
                        nc.scalar.activation(out=sg[:, a:b], in_=pgm[h][0:64, co:co + KT[k]],
                                             func=Act.Sigmoid)
                        nc.vector.tensor_tensor(out=h_in[:, a:b], in0=sg[:, a:b],
                                                in1=pgm[h][64:128, co:co + KT[k]], op=Alu.mult)
                        nc.vector.tensor_tensor(out=h_in[:, a:b], in0=h_in[:, a:b],
                                                in1=t_c15[:, a:b], op=Alu.add)
                        nc.tensor.transpose(pt[0:KT[k], 64 * k:64 * k + 64],
                                            h_in[:, a:b], t_eye[:, :])
                        nc.vector.tensor_copy(out=hT[0:KT[k], 64 * k:64 * k + 64],
                                              in_=pt[0:KT[k], 64 * k:64 * k + 64])

                # ---- deferred softmax for the previous step's logits ----
                if pending_enc is not None:
                    emit_enc(*pending_enc)
                    pending_enc = None

                # ---- GRU ----
                if idx > 0:
                    nc.scalar.dma_start(out=t_hT[118:125, 192:256],
                                        in_=t_xT[0:7, BC * idx:BC * idx + 64])
                prh = [pp.tile([128, 512], f32, tag="prhA", name="prhA"),
                       pp.tile([128, 512], f32, tag="prhB", name="prhB")]
                rp = hh_pairs(hT, 0)
                hp = hh_pairs(hT, 2)
                paired_half_mms(prh, rp, hp,
                                lambda k, c0, c1: t_whh[0:KTL[k], H3 * k + c0:H3 * k + c1],
                                lambda k, c0, c1: t_whh[0:KTL[k], H3 * k + 2 * H + c0:H3 * k + 2 * H + c1])
                pui = [pp.tile([128, 512], f32, tag="puiA", name="puiA"),
                       pp.tile([128, 512], f32, tag="puiB", name="puiB")]
                xsl = t_xT[0:8, BC * idx:BC * idx + 64]
                up = hh_pairs(hT, 1)
                for h, (c0, c1) in enumerate(((0, HA), (HA, H))):
                    w = c1 - c0
                    nc.tensor.matmul(pui[h][64:128, 0:w], xsl,
                                     t_wih[0:8, 2 * H + c0:2 * H + c1],
                                     start=True, stop=True, skip_group_check=True)
                    for k in range(4):
                        nc.tensor.matmul(pui[h][0:64, 0:w], up[k][0],
                                         t_whh[0:KTL[k], H3 * k + H + c0:H3 * k + H + c1],
                                         start=(k == 0), stop=(k == 3),
                                         skip_group_check=True)

                # merged GRU wave per K-chunk: r -> t1 -> t2 -> tanh -> d,
                # u -> hv = n + u*d, then transpose + aT-scale immediately.
                # Interleaving u with the tanh chain keeps sigma(u_k) from
                # queueing behind all four tanh ops on ACT.
                r = wp.tile([64, H], f32, tag="r")
                t1 = wp.tile([64, H], f32, tag="t1")
                t2 = wp.tile([64, H], f32, tag="t2")
                n = wp.tile([64, H], f32, tag="n")
                dd = wp.tile([64, H], f32, tag="dd")
                u = wp.tile([64, H], f32, tag="u")
                hv = wp.tile([64, H], f32, tag="hv")
                pt2 = pp.tile([128, 256], f32, tag="pT")
                gsT_new = gs_bufs[(idx + 1) % 3]
                for k in range(4):
                    a, b = CH[k]
                    h, co = half_of(k)
                    cs = slice(co, co + KT[k])
                    nc.scalar.activation(out=r[:, a:b], in_=prh[h][0:64, cs], func=Act.Sigmoid)
                    nc.vector.tensor_tensor(out=t1[:, a:b], in0=r[:, a:b],
                                            in1=prh[h][64:128, cs], op=Alu.mult)
                    nc.vector.tensor_tensor(out=t2[:, a:b], in0=t1[:, a:b],
                                            in1=pui[h][64:128, cs], op=Alu.add)
                    nc.scalar.activation(out=n[:, a:b], in_=t2[:, a:b], func=Act.Tanh)
                    nc.vector.tensor_tensor(out=dd[:, a:b], in0=h_in[:, a:b],
                                            in1=n[:, a:b], op=Alu.subtract)
                    nc.scalar.activation(out=u[:, a:b], in_=pui[h][0:64, cs], func=Act.Sigmoid)
                    nc.vector.tensor_tensor(out=hv[:, a:b], in0=u[:, a:b],
                                            in1=dd[:, a:b], op=Alu.mult)
                    nc.vector.tensor_tensor(out=hv[:, a:b], in0=hv[:, a:b],
                                            in1=n[:, a:b], op=Alu.add)
                    nc.tensor.transpose(pt2[0:KT[k], 64 * k:64 * k + 64],
                                        hv[:, a:b], t_eye[:, :])
                    nc.vector.tensor_copy(out=gsT_new[0:KT[k], 64 * k:64 * k + 64],
                                          in_=pt2[0:KT[k], 64 * k:64 * k + 64])
                    if idx + 1 < S:
                        s0 = 256 * idx
                        nc.vector.tensor_tensor(
                            out=t_aT[0:KT[k], 64 * k:64 * k + 64],
                            in0=pt2[0:KT[k], 64 * k:64 * k + 64],
                            in1=t_s4[0:KT[k], s0 + 64 * k:s0 + 64 * k + 64], op=Alu.mult)

                # ---- merged dots + next-step logits: [Wvert | w1 | w2] (N=9)
                # lhsT = hv_idx^T serves BOTH d1/d2[idx] and enc logits for idx+1
                psm = pp.tile([64, 16], f32, tag="psml")
                mm_group(psm[:, 0:9], w_pairs(gsT_new, t_wv, 9))
                nc.vector.tensor_copy(out=t_d1[:, idx:idx + 1], in_=psm[:, 7:8])
                nc.vector.tensor_copy(out=t_d2[:, idx:idx + 1], in_=psm[:, 8:9])
                if idx + 1 < S:
                    pending_enc = (idx + 1, psm)

                # ---- edge row idx: margins -> threshold -> mask (progressive) ----
                nc.vector.tensor_scalar(out=t_ed[:, 16 * idx:16 * idx + 16], in0=t_d2[:, :],
                                        scalar1=t_d1[:, idx:idx + 1], scalar2=None, op0=Alu.add)
                nc.vector.tensor_scalar(out=t_th[:, 16 * idx:16 * idx + 16],
                                        in0=t_ed[:, 16 * idx:16 * idx + 16],
                                        scalar1=t_nbe[:, :], scalar2=None, op0=Alu.is_ge)
                nc.vector.tensor_tensor(out=t_th[:, 16 * idx:16 * idx + 16],
                                        in0=t_th[:, 16 * idx:16 * idx + 16],
                                        in1=t_mask[:, 16 * idx:16 * idx + 16], op=Alu.mult)
                if idx >= 1:
                    # diagonal entry tth[:, 17*idx-1] = step(d1[idx-1]+d2[idx-1]+be)
                    nc.vector.tensor_tensor(out=t_sd[:, idx - 1:idx],
                                            in0=t_d1[:, idx - 1:idx],
                                            in1=t_d2[:, idx - 1:idx], op=Alu.add)
                    nc.vector.tensor_scalar(out=t_th[:, 17 * idx - 1:17 * idx],
                                            in0=t_sd[:, idx - 1:idx],
                                            scalar1=t_nbe[:, :], scalar2=None, op0=Alu.is_ge)

                gsT_prev = gsT_new

            if pending_enc is not None:
                emit_enc(*pending_enc)

            # edges were finalized progressively inside the step loop
            nc.sync.dma_start(out=d_odep.ap(), in_=t_th[:, :])
            nc.sync.dma_start(out=d_oenc.ap(), in_=t_enc[:, :])

    nc.compile()
    return nc


def _host_prep(z, dep_graph, node_encoding, W_lin1, b_lin1, W_vert, b_vert,
               W_edge, b_edge, W_gate, b_gate, W_map, b_map, W_ih, b_ih, W_hh, b_hh):
    f = np.float32

    def ktiled(WT, bias=None):  # WT: (H, N) -> (128, 4N); bias lands at row 117 of tile 3
        N = WT.shape[1]
        out = np.zeros((128, 4 * N), f)
        for k in range(4):
            out[0:KT[k], N * k:N * (k + 1)] = WT[KOFF[k]:KOFF[k] + KT[k], :]
        if bias is not None:
            out[117, 3 * N:3 * N + N] = bias
        return out

    whhT = np.zeros((128, 4 * H3), f)
    whhsrc = W_hh.T.astype(f)  # (H, 3H)
    for k in range(4):
        whhT[0:KT[k], H3 * k:H3 * (k + 1)] = whhsrc[KOFF[k]:KOFF[k] + KT[k], :]
    whhT[117, 3 * H3:4 * H3] = b_hh  # r/u thirds get b_ih added on-device
    # spare rows 118:125 of K-tile 3 carry W_ih.T for the r/u thirds (x rows of lhsT)
    whhT[118:125, 3 * H3:3 * H3 + 2 * H] = W_ih.T[:, 0:2 * H]

    shared = {
        "WlinT": ktiled(W_lin1.T.astype(f), b_lin1),
        "WgT": ktiled(W_gate.T.astype(f), b_gate),
        "WmT": ktiled(W_map.T.astype(f), b_map),
        "WhhT": whhT,
        "WvT": ktiled(np.concatenate([W_vert.T.astype(f),
                                      W_edge.reshape(2, H).T.astype(f)], axis=1),
                      np.concatenate([np.asarray(b_vert, f), np.zeros(2, f)])),
        "eye64": np.eye(64, dtype=f),
        "onesrow": np.ones((1, 64), f),
    }
    wih = np.zeros((8, H3), f)
    wih[0:7] = W_ih.T
    wih[7] = b_ih
    shared["WihT"] = wih
    mask = np.zeros((64, 256), f)
    for i in range(16):
        for j in range(16):
            if j <= i - 2:
                mask[:, 16 * i + j] = 1.0
    shared["maskOD"] = mask
    bias = np.zeros((1, BIAS_LEN), f)
    bias[0, BO_GATE:BO_GATE + H] = b_gate
    bias[0, BO_MAP:BO_MAP + H] = b_map
    bias[0, BO_BE] = np.asarray(b_edge).ravel()[0]
    bias[0, BO_IHRU:BO_IHRU + 2 * H] = b_ih[0:2 * H]
    bias[0, BO_HHRU:BO_HHRU + 2 * H] = b_hh[0:2 * H]
    bias[0, BO_IHN:BO_IHN + H] = b_ih[2 * H:3 * H]
    shared["BIASROW"] = bias
    shared["zero8"] = np.zeros((8, 64), f)

    in_maps = []
    for c in range(NCORES):
        sl = slice(c * BC, (c + 1) * BC)
        zc = np.asarray(z[sl], f)
        zt4 = np.zeros((128, 256), f)
        for k in range(4):
            zt4[0:KT[k], 64 * k:64 * k + 64] = zc[:, KOFF[k]:KOFF[k] + KT[k]].T
        zt4[117, 192:256] = 1.0  # ones row for b_lin1
        xt = np.zeros((8, S * BC), f)
        nec = np.asarray(node_encoding[sl], f)  # (64, S, C)
        for idx in range(S):
            xt[0:7, BC * idx:BC * (idx + 1)] = nec[:, idx, :].T
        xt[7] = 1.0
        depc = np.asarray(dep_graph[sl], f)
        s4 = np.zeros((128, 15 * 256), f)
        for i in range(1, S):
            sub = depc[:, i, i - 1]  # (64,)
            s4[:, 256 * (i - 1):256 * i] = np.tile(sub[None, :], (128, 4))
        m = dict(shared)
        m.update({"zT4": zt4, "xT": xt, "S4r": s4})
        in_maps.append(m)
    return in_maps


def kernel(**inputs):
    from concourse.bass_utils import run_bass_kernel_spmd

    if "nc" not in _CACHE:
        _CACHE["nc"] = _build_module()
    nc = _CACHE["nc"]
    in_maps = _host_prep(**inputs)
    res = run_bass_kernel_spmd(nc, in_maps, core_ids=list(range(NCORES)))
    dep_out = np.concatenate(
        [res.results[c]["out_dep"].reshape(BC, S, S) for c in range(NCORES)], axis=0)
    enc_out = np.concatenate(
        [res.results[c]["out_enc"].reshape(BC, S, C) for c in range(NCORES)], axis=0)
    return dep_out.astype(np.float32), enc_out.astype(np.float32)
